# revision 12
# baseline (speedup 1.0000x reference)
"""AtomAttention Bass kernel for Trainium2, 8-core data-parallel over B.

Per-molecule block-diagonal self-attention with mean aggregation:
  Q = xWq+bq ; K = xWk+bk ; V = xWv+bv
  S = QK^T * scale (masked per-molecule by lengths) ; P = softmax(S)
  out[b] = sum_valid_q (P @ V) / len_b

Algebraic restructuring used here:
  - softmax(S) is invariant to per-query constants, so
      S_eff = scale * (X A X^T + 1*(c^T))  with A = Wq Wk^T, c = X (Wk bq)
    (the (X Wq) bk^T and bq.bk terms are per-query constants -> dropped).
  - out[b] = w^T (X Wv) + bv  with w[k] = sum_{q valid} P[q,k] / len_b
    (since sum_k w[k] == 1 exactly, bv is added on the host at the end).

Device work per core (256 molecules, grouped 8 molecules = 512 rows):
  G^T = A^T X^T and V = X Wv  (f32r matmuls, moving dim 512)
  per pair of molecules (128 rows): S-block [128,128] via G^T/X^T slices,
  mask+bias via one K=3 matmul, exp+rowsum fused on ScalarE, w via one
  matmul with rhs = valid/len * 1/Z, out pair-block via W8^T V.
"""

import numpy as np

B, M, D = 2048, 64, 512
NCORES = 8
BC = B // NCORES      # 256 molecules per core
PAIRS = BC // 2       # 128
NG = 32               # groups of GM molecules per core
GM = 8                # molecules per group (512 rows)
CH = D // 128         # 4 contraction chunks
SCALE = 1.0 / float(np.sqrt(np.float32(D)))
NEG = -1.0e9

_CACHE: dict = {}


def _build(n_groups: int = NG, reps: int = 1):
    import concourse.mybir as mybir
    import concourse.tile as tile
    from concourse import bacc
    from concourse.bass import ts

    f32 = mybir.dt.float32
    f32r = mybir.dt.float32r
    AF = mybir.ActivationFunctionType

    nc = bacc.Bacc("TRN2", target_bir_lowering=False, debug=False)
    xt = nc.declare_dram_parameter("xt", [n_groups, CH, 128, 512], f32r, False)
    aw = nc.declare_dram_parameter("aw", [128, CH, 512], f32r, False)
    wv = nc.declare_dram_parameter("wv", [128, CH, 512], f32r, False)
    m3r = nc.declare_dram_parameter("m3r", [n_groups, 3, 4, 256], f32r, False)
    m3l = nc.declare_dram_parameter("m3l", [3, 128], f32r, False)
    vq = nc.declare_dram_parameter("vq", [128, n_groups, 4], f32, False)
    out = nc.declare_dram_parameter("out", [n_groups * GM, 512], f32, True)

    with tile.TileContext(nc) as tc:
        with (
            tc.tile_pool(name="singles", bufs=1) as singles,
            tc.tile_pool(name="xgp", bufs=3) as xgp,
            tc.tile_pool(name="gtp", bufs=2) as gtp,
            tc.tile_pool(name="vvp", bufs=2) as vvp,
            tc.tile_pool(name="m3p", bufs=2) as m3p,
            tc.tile_pool(name="prp", bufs=4) as prp,
            tc.tile_pool(name="smp", bufs=4) as smp,
            tc.tile_pool(name="osp", bufs=2) as osp,
            tc.tile_pool(name="psA", bufs=2, space="PSUM") as psA,
            tc.tile_pool(name="psS", bufs=2, space="PSUM") as psS,
            tc.tile_pool(name="psW", bufs=1, space="PSUM") as psW,
            tc.tile_pool(name="psO", bufs=1, space="PSUM") as psO,
        ):
            a_sb = singles.tile([128, CH, 512], f32r)
            nc.sync.dma_start(out=a_sb, in_=aw.ap())
            wv_sb = singles.tile([128, CH, 512], f32r)
            nc.sync.dma_start(out=wv_sb, in_=wv.ap())
            m3l_sb = singles.tile([3, 128], f32r)
            nc.sync.dma_start(out=m3l_sb, in_=m3l.ap())
            vq_sb = singles.tile([128, n_groups, 4], f32)
            nc.sync.dma_start(out=vq_sb, in_=vq.ap())

            # Two persistent W8 tiles (alternating per group). Only the
            # nonzero slivers are rewritten each group; zero once via the
            # ACT scale=0 trick (memset does not support f32r).
            w8_a = singles.tile([128, CH, 8], f32r)
            w8_b = singles.tile([128, CH, 8], f32r)
            w8_tiles = [w8_a, w8_b]
            for t in w8_tiles:
                nc.scalar.activation(
                    t, a_sb[:, 0:CH, 0:8], AF.Copy, scale=0.0, bias=0.0
                )

            def body():
                for g in range(n_groups):
                    xg = xgp.tile([128, CH, 512], f32r, tag="xg")
                    for c in range(CH):
                        nc.sync.dma_start(out=xg[:, c, :], in_=xt.ap()[g, c])
                    m3g = m3p.tile([3, 4, 256], f32r, tag="m3g")
                    nc.sync.dma_start(out=m3g, in_=m3r.ap()[g])

                    # G^T[e, r] = sum_d A[d, e] X^T[d, r]
                    gt = gtp.tile([128, CH, 512], f32r, tag="gt")
                    for ec in range(CH):
                        gps = psA.tile([128, 512], f32, tag="gps")
                        for c in range(CH):
                            nc.tensor.matmul(
                                gps,
                                a_sb[:, c, ts(ec, 128)],
                                xg[:, c, :],
                                start=(c == 0),
                                stop=(c == CH - 1),
                            )
                        nc.scalar.copy(gt[:, ec, :], gps)

                    # V[r, e] = sum_d X^T[d, r] Wv[d, e]
                    vv = vvp.tile([128, CH, 512], f32r, tag="vv")
                    for rt in range(CH):
                        vps = psA.tile([128, 512], f32, tag="vps")
                        for c in range(CH):
                            nc.tensor.matmul(
                                vps,
                                xg[:, c, ts(rt, 128)],
                                wv_sb[:, c, :],
                                start=(c == 0),
                                stop=(c == CH - 1),
                            )
                        nc.vector.tensor_copy(vv[:, rt, :], vps)

                    w8 = w8_tiles[g % 2]
                    for j in range(4):
                        # S[q, k] over the duo window (256 keys) so the f32r
                        # matmul streams at 1 cycle/row (needs moving dim>=256).
                        t2 = j // 2  # duo index within group
                        sps = psS.tile([128, 256], f32, tag="sps")
                        for c2 in range(CH):
                            nc.tensor.matmul(
                                sps,
                                gt[:, c2, ts(j, 128)],
                                xg[:, c2, ts(t2, 256)],
                                start=(c2 == 0),
                                stop=False,
                            )
                        nc.tensor.matmul(
                            sps,
                            m3l_sb,
                            m3g[:, j, :],
                            start=False,
                            stop=True,
                        )
                        probs = prp.tile([128, 256], f32r, tag="probs")
                        zz = smp.tile([128, 1], f32, tag="zz")
                        nc.scalar.activation(
                            probs, sps, AF.Exp, scale=SCALE, accum_out=zz
                        )
                        rz = smp.tile([128, 1], f32, tag="rz")
                        nc.vector.reciprocal(rz, zz)
                        vqz = smp.tile([128, 2], f32r, tag="vqz")
                        nc.vector.tensor_mul(vqz[:, 0:1], rz, vq_sb[:, g, j : j + 1])
                        nc.vector.tensor_mul(vqz[:, 1:2], rz, vq_sb[:, g, j : j + 1])
                        wps = psW.tile([128, 2], f32, tag="wps")
                        nc.tensor.matmul(
                            wps,
                            probs[:, ts(j % 2, 128)],
                            vqz,
                            start=True,
                            stop=True,
                        )
                        nc.vector.tensor_copy(
                            w8[0:64, j, 2 * j : 2 * j + 1], wps[0:64, 0:1]
                        )
                        nc.vector.tensor_copy(
                            w8[64:128, j, 2 * j + 1 : 2 * j + 2], wps[64:128, 0:1]
                        )

                    # out[m, e] = sum_rows W8[row, m] V[row, e]
                    ops_ = psO.tile([8, 512], f32, tag="ops")
                    for c3 in range(CH):
                        nc.tensor.matmul(
                            ops_,
                            w8[:, c3, :],
                            vv[:, c3, :],
                            start=(c3 == 0),
                            stop=(c3 == CH - 1),
                        )
                    osb = osp.tile([8, 512], f32, tag="osb")
                    nc.scalar.copy(osb, ops_)
                    nc.sync.dma_start(out=out.ap()[g * GM : (g + 1) * GM, :], in_=osb)

            if reps == 1:
                body()
            else:
                with tc.For_i(0, reps, 1):
                    body()
    nc.compile()
    return nc


def _host_prep(x, lengths, Wq, bq, Wk, bk, Wv, bv, n_groups: int = NG):
    """Builds per-core input maps. Returns (in_maps, bv) with bv to add on host."""
    f32 = np.float32
    x = np.ascontiguousarray(np.asarray(x, f32))
    lengths = np.asarray(lengths, np.int32)
    Wq = np.asarray(Wq, f32)
    bq = np.asarray(bq, f32)
    Wk = np.asarray(Wk, f32)
    Wv = np.asarray(Wv, f32)

    A = (Wq.astype(np.float64) @ Wk.T.astype(np.float64)).astype(f32)
    u = (Wk.astype(np.float64) @ bq.astype(np.float64)).astype(f32)
    cvec = (x.reshape(-1, D) @ u).reshape(x.shape[0], M)
    valid = np.arange(M)[None, :] < lengths[:, None]
    maskc = np.where(valid, cvec, f32(NEG)).astype(f32)
    vqdiv = (valid / lengths[:, None]).astype(f32)

    aw_h = np.ascontiguousarray(A.reshape(CH, 128, D).transpose(1, 0, 2))
    wv_h = np.ascontiguousarray(Wv.reshape(CH, 128, D).transpose(1, 0, 2))
    u0 = (np.arange(128) < 64).astype(f32)
    u1 = f32(1.0) - u0
    m3l_h = np.ascontiguousarray(np.stack([np.ones(128, f32), u0, u1]))

    bc = n_groups * GM
    n_cores = x.shape[0] // bc
    npairs = bc // 2
    in_maps = []
    for ci in range(n_cores):
        xc = x[ci * bc : (ci + 1) * bc]
        xt_h = np.ascontiguousarray(
            xc.reshape(n_groups, GM, M, CH, 128)
            .transpose(0, 3, 4, 1, 2)
            .reshape(n_groups, CH, 128, GM * M)
        )
        mcd = maskc[ci * bc : (ci + 1) * bc].reshape(npairs // 2, 256)
        m3r_h = np.empty((npairs, 3, 256), f32)
        m3r_h[:, 0] = np.repeat(mcd, 2, axis=0)
        blk = np.full((4, 256), f32(NEG), f32)
        for i in range(4):
            blk[i, i * 64 : (i + 1) * 64] = 0.0
        m3r_h[0::2, 1] = blk[0]
        m3r_h[0::2, 2] = blk[1]
        m3r_h[1::2, 1] = blk[2]
        m3r_h[1::2, 2] = blk[3]
        m3r_h = np.ascontiguousarray(
            m3r_h.reshape(n_groups, 4, 3, 256).transpose(0, 2, 1, 3)
        )
        vq_h = np.ascontiguousarray(
            vqdiv[ci * bc : (ci + 1) * bc].reshape(npairs, 128).T.reshape(
                128, n_groups, 4
            )
        )
        in_maps.append(
            {"xt": xt_h, "aw": aw_h, "wv": wv_h, "m3r": m3r_h, "m3l": m3l_h, "vq": vq_h}
        )
    return in_maps


def _make_runner(nc):
    """One-time jit of the 8-core shard_map around the bass custom call.

    Returns run(in_maps) -> [per-core out arrays]. Reusing the jitted
    function across calls avoids per-call retrace/recompile.
    """
    import jax
    import numpy as np_
    from jax.sharding import Mesh, PartitionSpec
    from jax.experimental.shard_map import shard_map
    import concourse.mybir as mybir
    from concourse import bass2jax
    from concourse.bass2jax import _bass_exec_p, install_neuronx_cc_hook

    install_neuronx_cc_hook()

    in_names, out_names, out_avals, out_shapes = [], [], [], []
    partition_name = nc.partition_id_tensor.name if nc.partition_id_tensor else None
    for alloc in nc.m.functions[0].allocations:
        if not isinstance(alloc, mybir.MemoryLocationSet):
            continue
        name = alloc.memorylocations[0].name
        if alloc.kind == "ExternalInput":
            if name != partition_name:
                in_names.append(name)
        elif alloc.kind == "ExternalOutput":
            shape = tuple(alloc.tensor_shape)
            dtype = mybir.dt.np(alloc.dtype)
            out_names.append(name)
            out_shapes.append((shape, dtype))
            out_avals.append(jax.core.ShapedArray(shape, dtype))
    n_params = len(in_names)
    all_in_names = in_names + out_names + ([partition_name] if partition_name else [])

    def _body(*args):
        operands = list(args)
        if partition_name is not None:
            operands.append(bass2jax.partition_id_tensor())
        outs = _bass_exec_p.bind(
            *operands,
            out_avals=tuple(out_avals),
            in_names=tuple(all_in_names),
            out_names=tuple(out_names),
            lowering_input_output_aliases=(),
            sim_require_finite=True,
            sim_require_nnan=True,
            nc=nc,
        )
        return tuple(outs)

    devices = jax.devices()[:NCORES]
    mesh = Mesh(np_.asarray(devices), ("core",))
    in_specs = (PartitionSpec("core"),) * (n_params + len(out_names))
    out_specs = (PartitionSpec("core"),) * len(out_names)
    fn = jax.jit(
        shard_map(
            _body, mesh=mesh, in_specs=in_specs, out_specs=out_specs, check_rep=False
        ),
        keep_unused=True,
    )
    zero_concat = [
        np_.zeros((NCORES * s[0], *s[1:]), dt) for s, dt in out_shapes
    ]

    def run(in_maps):
        concat_in = [
            np_.concatenate([in_maps[c][n] for c in range(NCORES)], axis=0)
            for n in in_names
        ]
        outs = fn(*concat_in, *zero_concat)
        o = np_.asarray(outs[0])
        per_core_rows = out_shapes[0][0][0]
        return o.reshape(NCORES, per_core_rows, *out_shapes[0][0][1:])

    return run


def kernel(x, lengths, Wq, bq, Wk, bk, Wv, bv) -> np.ndarray:
    if "runner" not in _CACHE:
        nc = _build(NG, 1)
        _CACHE["runner"] = _make_runner(nc)
    run = _CACHE["runner"]

    in_maps = _host_prep(x, lengths, Wq, bq, Wk, bk, Wv, bv)
    outs = run(in_maps).reshape(B, D)
    bv = np.asarray(bv, np.float32)
    return (outs + bv[None, :]).astype(np.float32)


# revision 14
# speedup vs baseline: 2.2631x; 2.2631x over previous
"""AtomAttention Bass kernel for Trainium2, 8-core data-parallel over B.

Per-molecule block-diagonal self-attention with mean aggregation:
  Q = xWq+bq ; K = xWk+bk ; V = xWv+bv
  S = QK^T * scale (masked per-molecule by lengths) ; P = softmax(S)
  out[b] = sum_valid_q (P @ V) / len_b

Algebraic restructuring used here:
  - softmax(S) is invariant to per-query constants, so
      S_eff = scale * (X A X^T + 1*(c^T))  with A = Wq Wk^T, c = X (Wk bq)
    (the (X Wq) bk^T and bq.bk terms are per-query constants -> dropped).
  - out[b] = w^T (X Wv) + bv  with w[k] = sum_{q valid} P[q,k] / len_b
    (since sum_k w[k] == 1 exactly, bv is added on the host at the end).

Device work per core (256 molecules, grouped 8 molecules = 512 rows):
  G^T = A^T X^T and V = X Wv  (f32r matmuls, moving dim 512)
  per pair of molecules (128 rows): S-block [128,128] via G^T/X^T slices,
  mask+bias via one K=3 matmul, exp+rowsum fused on ScalarE, w via one
  matmul with rhs = valid/len * 1/Z, out pair-block via W8^T V.
"""

import numpy as np

B, M, D = 2048, 64, 512
NCORES = 8
BC = B // NCORES      # 256 molecules per core
PAIRS = BC // 2       # 128
NG = 32               # groups of GM molecules per core
GM = 8                # molecules per group (512 rows)
CH = D // 128         # 4 contraction chunks
SCALE = 1.0 / float(np.sqrt(np.float32(D)))
NEG = -1.0e9

_CACHE: dict = {}


def _build(n_groups: int = NG, reps: int = 1, mode: str = "full"):
    import concourse.mybir as mybir
    import concourse.tile as tile
    from concourse import bacc
    from concourse.bass import ts

    f32 = mybir.dt.float32
    f32r = mybir.dt.float32r
    AF = mybir.ActivationFunctionType

    nc = bacc.Bacc("TRN2", target_bir_lowering=False, debug=False)
    xt = nc.declare_dram_parameter("xt", [n_groups, CH, 128, 512], f32r, False)
    aw = nc.declare_dram_parameter("aw", [128, CH, 512], f32r, False)
    wv = nc.declare_dram_parameter("wv", [128, CH, 512], f32r, False)
    m3r = nc.declare_dram_parameter("m3r", [n_groups, 3, 4, 256], f32r, False)
    m3l = nc.declare_dram_parameter("m3l", [3, 128], f32r, False)
    vq = nc.declare_dram_parameter("vq", [128, n_groups, 4], f32, False)
    out = nc.declare_dram_parameter("out", [n_groups * GM, 512], f32, True)

    with tile.TileContext(nc) as tc:
        with (
            tc.tile_pool(name="singles", bufs=1) as singles,
            tc.tile_pool(name="xgp", bufs=3) as xgp,
            tc.tile_pool(name="gtp", bufs=2) as gtp,
            tc.tile_pool(name="vvp", bufs=2) as vvp,
            tc.tile_pool(name="m3p", bufs=2) as m3p,
            tc.tile_pool(name="prp", bufs=4) as prp,
            tc.tile_pool(name="smp", bufs=4) as smp,
            tc.tile_pool(name="osp", bufs=2) as osp,
            tc.tile_pool(name="psA", bufs=2, space="PSUM") as psA,
            tc.tile_pool(name="psS", bufs=2, space="PSUM") as psS,
            tc.tile_pool(name="psW", bufs=1, space="PSUM") as psW,
            tc.tile_pool(name="psO", bufs=1, space="PSUM") as psO,
        ):
            a_sb = singles.tile([128, CH, 512], f32r)
            nc.sync.dma_start(out=a_sb, in_=aw.ap())
            wv_sb = singles.tile([128, CH, 512], f32r)
            nc.sync.dma_start(out=wv_sb, in_=wv.ap())
            m3l_sb = singles.tile([3, 128], f32r)
            nc.sync.dma_start(out=m3l_sb, in_=m3l.ap())
            vq_sb = singles.tile([128, n_groups, 4], f32)
            nc.sync.dma_start(out=vq_sb, in_=vq.ap())

            # Two persistent W8 tiles (alternating per group). Only the
            # nonzero slivers are rewritten each group; zero once via the
            # ACT scale=0 trick (memset does not support f32r).
            w8_a = singles.tile([128, CH, 8], f32r)
            w8_b = singles.tile([128, CH, 8], f32r)
            w8_tiles = [w8_a, w8_b]
            for t in w8_tiles:
                nc.scalar.activation(
                    t, a_sb[:, 0:CH, 0:8], AF.Copy, scale=0.0, bias=0.0
                )

            def body():
                for g in range(n_groups):
                    xg = xgp.tile([128, CH, 512], f32r, tag="xg")
                    for c in range(CH):
                        nc.sync.dma_start(out=xg[:, c, :], in_=xt.ap()[g, c])
                    m3g = m3p.tile([3, 4, 256], f32r, tag="m3g")
                    nc.sync.dma_start(out=m3g, in_=m3r.ap()[g])

                    # G^T[e, r] = sum_d A[d, e] X^T[d, r]
                    gt = gtp.tile([128, CH, 512], f32r, tag="gt")
                    for ec in range(CH):
                        gps = psA.tile([128, 512], f32, tag="gps")
                        for c in range(CH):
                            nc.tensor.matmul(
                                gps,
                                a_sb[:, c, ts(ec, 128)],
                                xg[:, c, :],
                                start=(c == 0),
                                stop=(c == CH - 1),
                            )
                        nc.scalar.copy(gt[:, ec, :], gps)

                    # V[r, e] = sum_d X^T[d, r] Wv[d, e]
                    vv = vvp.tile([128, CH, 512], f32r, tag="vv")
                    for rt in range(CH):
                        vps = psA.tile([128, 512], f32, tag="vps")
                        for c in range(CH):
                            nc.tensor.matmul(
                                vps,
                                xg[:, c, ts(rt, 128)],
                                wv_sb[:, c, :],
                                start=(c == 0),
                                stop=(c == CH - 1),
                            )
                        nc.vector.tensor_copy(vv[:, rt, :], vps)

                    if mode == "proj":
                        continue
                    w8 = w8_tiles[g % 2]
                    for j in range(4) if mode == "full" else []:
                        # S[q, k] over the duo window (256 keys) so the f32r
                        # matmul streams at 1 cycle/row (needs moving dim>=256).
                        t2 = j // 2  # duo index within group
                        sps = psS.tile([128, 256], f32, tag="sps")
                        for c2 in range(CH):
                            nc.tensor.matmul(
                                sps,
                                gt[:, c2, ts(j, 128)],
                                xg[:, c2, ts(t2, 256)],
                                start=(c2 == 0),
                                stop=False,
                            )
                        nc.tensor.matmul(
                            sps,
                            m3l_sb,
                            m3g[:, j, :],
                            start=False,
                            stop=True,
                        )
                        probs = prp.tile([128, 256], f32r, tag="probs")
                        zz = smp.tile([128, 1], f32, tag="zz")
                        nc.scalar.activation(
                            probs, sps, AF.Exp, scale=SCALE, accum_out=zz
                        )
                        rz = smp.tile([128, 1], f32, tag="rz")
                        nc.vector.reciprocal(rz, zz)
                        vqz = smp.tile([128, 2], f32r, tag="vqz")
                        nc.vector.tensor_mul(vqz[:, 0:1], rz, vq_sb[:, g, j : j + 1])
                        nc.vector.tensor_mul(vqz[:, 1:2], rz, vq_sb[:, g, j : j + 1])
                        wps = psW.tile([128, 2], f32, tag="wps")
                        nc.tensor.matmul(
                            wps,
                            probs[:, ts(j % 2, 128)],
                            vqz,
                            start=True,
                            stop=True,
                        )
                        nc.vector.tensor_copy(
                            w8[0:64, j, 2 * j : 2 * j + 1], wps[0:64, 0:1]
                        )
                        nc.vector.tensor_copy(
                            w8[64:128, j, 2 * j + 1 : 2 * j + 2], wps[64:128, 0:1]
                        )

                    # out[m, e] = sum_rows W8[row, m] V[row, e]
                    ops_ = psO.tile([8, 512], f32, tag="ops")
                    for c3 in range(CH):
                        nc.tensor.matmul(
                            ops_,
                            w8[:, c3, :],
                            vv[:, c3, :],
                            start=(c3 == 0),
                            stop=(c3 == CH - 1),
                        )
                    osb = osp.tile([8, 512], f32, tag="osb")
                    nc.scalar.copy(osb, ops_)
                    nc.sync.dma_start(out=out.ap()[g * GM : (g + 1) * GM, :], in_=osb)

            if reps == 1:
                body()
            else:
                with tc.For_i(0, reps, 1):
                    body()
    nc.compile()
    return nc


def _host_prep(x, lengths, Wq, bq, Wk, bk, Wv, bv, n_groups: int = NG):
    """Builds per-core input maps. Returns (in_maps, bv) with bv to add on host."""
    f32 = np.float32
    x = np.ascontiguousarray(np.asarray(x, f32))
    lengths = np.asarray(lengths, np.int32)
    Wq = np.asarray(Wq, f32)
    bq = np.asarray(bq, f32)
    Wk = np.asarray(Wk, f32)
    Wv = np.asarray(Wv, f32)

    A = (Wq.astype(np.float64) @ Wk.T.astype(np.float64)).astype(f32)
    u = (Wk.astype(np.float64) @ bq.astype(np.float64)).astype(f32)
    cvec = (x.reshape(-1, D) @ u).reshape(x.shape[0], M)
    valid = np.arange(M)[None, :] < lengths[:, None]
    maskc = np.where(valid, cvec, f32(NEG)).astype(f32)
    vqdiv = (valid / lengths[:, None]).astype(f32)

    aw_h = np.ascontiguousarray(A.reshape(CH, 128, D).transpose(1, 0, 2))
    wv_h = np.ascontiguousarray(Wv.reshape(CH, 128, D).transpose(1, 0, 2))
    u0 = (np.arange(128) < 64).astype(f32)
    u1 = f32(1.0) - u0
    m3l_h = np.ascontiguousarray(np.stack([np.ones(128, f32), u0, u1]))

    bc = n_groups * GM
    n_cores = x.shape[0] // bc
    npairs = bc // 2
    in_maps = []
    for ci in range(n_cores):
        xc = x[ci * bc : (ci + 1) * bc]
        xt_h = np.ascontiguousarray(
            xc.reshape(n_groups, GM, M, CH, 128)
            .transpose(0, 3, 4, 1, 2)
            .reshape(n_groups, CH, 128, GM * M)
        )
        mcd = maskc[ci * bc : (ci + 1) * bc].reshape(npairs // 2, 256)
        m3r_h = np.empty((npairs, 3, 256), f32)
        m3r_h[:, 0] = np.repeat(mcd, 2, axis=0)
        blk = np.full((4, 256), f32(NEG), f32)
        for i in range(4):
            blk[i, i * 64 : (i + 1) * 64] = 0.0
        m3r_h[0::2, 1] = blk[0]
        m3r_h[0::2, 2] = blk[1]
        m3r_h[1::2, 1] = blk[2]
        m3r_h[1::2, 2] = blk[3]
        m3r_h = np.ascontiguousarray(
            m3r_h.reshape(n_groups, 4, 3, 256).transpose(0, 2, 1, 3)
        )
        vq_h = np.ascontiguousarray(
            vqdiv[ci * bc : (ci + 1) * bc].reshape(npairs, 128).T.reshape(
                128, n_groups, 4
            )
        )
        in_maps.append(
            {"xt": xt_h, "aw": aw_h, "wv": wv_h, "m3r": m3r_h, "m3l": m3l_h, "vq": vq_h}
        )
    return in_maps


def _make_runner(nc):
    """One-time jit of the 8-core shard_map around the bass custom call.

    Returns run(in_maps) -> [per-core out arrays]. Reusing the jitted
    function across calls avoids per-call retrace/recompile.
    """
    import jax
    import numpy as np_
    from jax.sharding import Mesh, PartitionSpec
    from jax.experimental.shard_map import shard_map
    import concourse.mybir as mybir
    from concourse import bass2jax
    from concourse.bass2jax import _bass_exec_p, install_neuronx_cc_hook

    install_neuronx_cc_hook()

    in_names, out_names, out_avals, out_shapes = [], [], [], []
    partition_name = nc.partition_id_tensor.name if nc.partition_id_tensor else None
    for alloc in nc.m.functions[0].allocations:
        if not isinstance(alloc, mybir.MemoryLocationSet):
            continue
        name = alloc.memorylocations[0].name
        if alloc.kind == "ExternalInput":
            if name != partition_name:
                in_names.append(name)
        elif alloc.kind == "ExternalOutput":
            shape = tuple(alloc.tensor_shape)
            dtype = mybir.dt.np(alloc.dtype)
            out_names.append(name)
            out_shapes.append((shape, dtype))
            out_avals.append(jax.core.ShapedArray(shape, dtype))
    n_params = len(in_names)
    all_in_names = in_names + out_names + ([partition_name] if partition_name else [])

    def _body(*args):
        operands = list(args)
        if partition_name is not None:
            operands.append(bass2jax.partition_id_tensor())
        outs = _bass_exec_p.bind(
            *operands,
            out_avals=tuple(out_avals),
            in_names=tuple(all_in_names),
            out_names=tuple(out_names),
            lowering_input_output_aliases=(),
            sim_require_finite=True,
            sim_require_nnan=True,
            nc=nc,
        )
        return tuple(outs)

    devices = jax.devices()[:NCORES]
    mesh = Mesh(np_.asarray(devices), ("core",))
    in_specs = (PartitionSpec("core"),) * (n_params + len(out_names))
    out_specs = (PartitionSpec("core"),) * len(out_names)
    fn = jax.jit(
        shard_map(
            _body, mesh=mesh, in_specs=in_specs, out_specs=out_specs, check_rep=False
        ),
        keep_unused=True,
    )
    zero_concat = [
        np_.zeros((NCORES * s[0], *s[1:]), dt) for s, dt in out_shapes
    ]

    def run(in_maps):
        concat_in = [
            np_.concatenate([in_maps[c][n] for c in range(NCORES)], axis=0)
            for n in in_names
        ]
        outs = fn(*concat_in, *zero_concat)
        o = np_.asarray(outs[0])
        per_core_rows = out_shapes[0][0][0]
        return o.reshape(NCORES, per_core_rows, *out_shapes[0][0][1:])

    return run


def kernel(x, lengths, Wq, bq, Wk, bk, Wv, bv) -> np.ndarray:
    if "runner" not in _CACHE:
        nc = _build(NG, 1)
        _CACHE["runner"] = _make_runner(nc)
    run = _CACHE["runner"]

    in_maps = _host_prep(x, lengths, Wq, bq, Wk, bk, Wv, bv)
    outs = run(in_maps).reshape(B, D)
    bv = np.asarray(bv, np.float32)
    return (outs + bv[None, :]).astype(np.float32)
